# revision 1
# baseline (speedup 1.0000x reference)
"""B-spline (clamped) surface evaluation on 8 Trainium2 cores.

Math: out[u, v, :] = sum_{a,b} Bu[u,a] * Bv[v,b] * P[su[u]-p+a, sv[v]-p+b, :]

Host precomputes the tiny Cox-de-Boor basis, scatters it into dense matrices
Au [Nu, 64], Av [Nv, 64], and folds the small control-point contraction
T[u, j, d] = sum_i Au[u, i] P[i, j, d] (25M MACs, fp64 on host). The device
then does the dominant contraction (768M MACs, 48 MB output):

  S[u, v, d] = sum_j T[u, j, d] * Av[v, j]       (TensorEngine matmuls)

fp32 matmul on TRN2 is ~5x slower than bf16, so operands are split
a = hi + lo (bf16 each) and the K=64 contraction is packed into the full
K=128 PE array: lhsT = [hi; lo] stacked on partitions; one matmul against
[b_hi; b_hi] plus one against [b_lo; b_lo] accumulating in PSUM computes the
exact (hi+lo)(b_hi+b_lo) product — 4-pass accuracy for 2 matmuls.

The device writes d-plane rows (out[u] = [S(u,:,0) | S(u,:,1) | S(u,:,2)]);
the host interleaves to [Nu, Nv, 3] while unsharding. PSUM->SBUF copies of
different d-planes touch disjoint byte ranges, so Tile runs them on
VectorE/ScalarE concurrently. All DMA goes through the gpsimd SWDGE path —
the HWDGE rings were observed draining through only 3 of 16 SDMA engines.

Sharding: data-parallel over u. Each core computes a [251, 2001, 3] slab.
"""

import numpy as np

N_CTRL = 64
N_EVAL = 2001
N_CORES = 8
NU_SHARD = 251  # ceil(2001 / 8); 8 * 251 = 2008 (last 7 rows are zero padding)
U_TILES = [(0, 128), (128, NU_SHARD - 128)]
V_TILE = 512
V_SPLIT = 2 * V_TILE  # column where the avt input (and output DMA) splits

_CACHE = {}


def _clamped_knots(p, n_ctrl, dtype=np.float64):
    n_internal = n_ctrl - p - 1
    internal = np.linspace(0.0, 1.0, n_internal + 2, dtype=dtype)[1:-1]
    return np.concatenate(
        [np.zeros(p + 1, dtype), internal, np.ones(p + 1, dtype)]
    )


def _dense_basis(params, p, n_ctrl):
    """Dense basis matrix A [len(params), n_ctrl], float64, with
    A[k, span-p+a] = B[k, a] (Cox-de-Boor, NURBS book A2.2)."""
    knots = _clamped_knots(p, n_ctrl)
    u = np.asarray(params, np.float64)
    spans = np.clip(np.searchsorted(knots, u, side="right") - 1, p, n_ctrl - 1)
    Ns = [np.ones_like(u)]
    left = {}
    right = {}
    for j in range(1, p + 1):
        left[j] = u - knots[spans + 1 - j]
        right[j] = knots[spans + j] - u
        saved = np.zeros_like(u)
        new = []
        for r in range(j):
            temp = Ns[r] / (right[r + 1] + left[j - r])
            new.append(saved + right[r + 1] * temp)
            saved = left[j - r] * temp
        new.append(saved)
        Ns = new
    B = np.stack(Ns, axis=-1)  # [N, p+1]
    A = np.zeros((len(u), n_ctrl), np.float64)
    rows = np.arange(len(u))[:, None]
    cols = spans[:, None] - p + np.arange(p + 1)[None, :]
    A[rows, cols] = B
    return A


def _split_bf16(a):
    """fp32 array -> (hi, lo) bf16 arrays with hi + lo ~= a (~2^-17 rel)."""
    import ml_dtypes

    a = np.ascontiguousarray(a, np.float32)
    hi = a.astype(ml_dtypes.bfloat16)
    lo = (a - hi.astype(np.float32)).astype(ml_dtypes.bfloat16)
    return hi, lo


def _stack_hilo(a):
    """[64, N] fp32 -> [128, N] bf16 with rows 0-63 = hi, 64-127 = lo."""
    hi, lo = _split_bf16(a)
    return np.ascontiguousarray(np.concatenate([hi, lo], axis=0))


def _dup_halves(a_bf16):
    """[64, N] bf16 -> [128, N] with the same data in both partition halves."""
    return np.ascontiguousarray(np.concatenate([a_bf16, a_bf16], axis=0))


def _build_device():
    if "nc" in _CACHE:
        return _CACHE["nc"]

    import concourse.mybir as mybir
    import concourse.tile as tile
    from concourse import bacc

    f32 = mybir.dt.float32
    bf16 = mybir.dt.bfloat16
    nc = bacc.Bacc(
        "TRN2", target_bir_lowering=False, debug=False, num_devices=N_CORES,
        num_swdge_queues=4,
    )
    # avt split into column halves so stage 2 can start before the full
    # matrix lands
    in_specs = [
        ("tt_p", 3 * NU_SHARD),       # [T_hi; T_lo], cols d*NU_SHARD + u
        ("avt_hi_a", V_SPLIT),        # [avt_hi; avt_hi], v < V_SPLIT
        ("avt_lo_a", V_SPLIT),
        ("avt_hi_b", N_EVAL - V_SPLIT),
        ("avt_lo_b", N_EVAL - V_SPLIT),
    ]
    ins = {
        name: nc.dram_tensor(name, [128, cols], bf16, kind="ExternalInput").ap()
        for name, cols in in_specs
    }
    # d-plane row layout: row u = [d0 | d1 | d2], each N_EVAL wide
    out_h = nc.dram_tensor(
        "out", [NU_SHARD, 3 * N_EVAL], f32, kind="ExternalOutput"
    ).ap()

    with tile.TileContext(nc) as tc:
        with (
            tc.tile_pool(name="consts", bufs=1) as consts,
            tc.tile_pool(name="ps2", bufs=6, space="PSUM") as ps2,
            tc.tile_pool(name="obuf", bufs=3) as obuf,
        ):
            sb = {}
            for name, cols in in_specs:
                sb[name] = consts.tile([128, cols], bf16, tag=name, name=name)
                nc.gpsimd.dma_start(out=sb[name], in_=ins[name])

            # S[u, v, d] = sum_j Tt_d[j, u] * Av[v, j]
            # One explicit LDWEIGHTS per (u-tile, d) group; the matmuls are
            # marked non-self-loading (ldweights=False) so the PE streams
            # back-to-back instead of reloading identical weights per matmul.
            # add_dep_helper pins the LDW <-> matmul ordering on the PE queue.
            n_copy = 0
            n_out = 0
            prev_mm = None
            for u0, uw in U_TILES:
                for d in range(3):
                    usl = slice(d * NU_SHARD + u0, d * NU_SHARD + u0 + uw)
                    w = sb["tt_p"][:, usl]
                    ldw = nc.tensor.ldweights(w)
                    if prev_mm is not None:
                        tile.add_dep_helper(
                            ldw.ins, prev_mm.ins, sync=False,
                            reason="weight group order",
                        )
                    ob = obuf.tile([128, N_EVAL], f32, tag="ob")
                    for v0 in range(0, N_EVAL, V_TILE):
                        vw = min(V_TILE, N_EVAL - v0)
                        half = "a" if v0 < V_SPLIT else "b"
                        hv0 = v0 if half == "a" else v0 - V_SPLIT
                        hsl = slice(hv0, hv0 + vw)
                        ps = ps2.tile([128, V_TILE], f32, tag="ps")
                        mm1 = nc.tensor.matmul(
                            ps[:uw, :vw], w, sb[f"avt_hi_{half}"][:, hsl],
                            start=True, stop=False,
                        )
                        mm2 = nc.tensor.matmul(
                            ps[:uw, :vw], w, sb[f"avt_lo_{half}"][:, hsl],
                            start=False, stop=True,
                        )
                        for mm in (mm1, mm2):
                            mm.ins.ldweights = False
                            tile.add_dep_helper(
                                mm.ins, ldw.ins, sync=False,
                                reason="matmul after its ldweights",
                            )
                        prev_mm = mm2
                        # alternate engines; different (d, vt) regions are
                        # disjoint, so DVE and ACT copies run concurrently
                        if n_copy % 2 == 0:
                            nc.vector.tensor_copy(
                                ob[:uw, v0 : v0 + vw], ps[:uw, :vw]
                            )
                        else:
                            nc.scalar.copy(ob[:uw, v0 : v0 + vw], ps[:uw, :vw])
                        n_copy += 1
                        # flush each ob half as soon as its copies are done.
                        # One SWDGE queue caps at ~3 concurrent DMAs x one
                        # engine-triple each (~230 GB/s); round-robin the
                        # output DMAs over 4 SWDGE queues to engage more of
                        # the 16 SDMA engines.
                        if v0 + vw in (V_SPLIT, N_EVAL):
                            f0 = 0 if v0 + vw == V_SPLIT else V_SPLIT
                            dma = nc.gpsimd.dma_start(
                                out=out_h[
                                    u0 : u0 + uw,
                                    d * N_EVAL + f0 : d * N_EVAL + v0 + vw,
                                ],
                                in_=ob[:uw, f0 : v0 + vw],
                            )
                            qi = n_out % 4
                            dma.ins.queue = f"qPoolDynamic{qi or ''}"
                            n_out += 1
    nc.compile()
    _CACHE["nc"] = nc
    return nc


def kernel(control_points, params_u, params_v, degree):
    from concourse.bass_utils import run_bass_kernel_spmd

    p = int(np.asarray(degree))
    cp = np.asarray(control_points, np.float32)
    pu = np.asarray(params_u, np.float32)
    pv = np.asarray(params_v, np.float32)
    assert cp.shape == (N_CTRL, N_CTRL, 3), cp.shape
    assert pu.shape == (N_EVAL,) and pv.shape == (N_EVAL,), (pu.shape, pv.shape)

    Au = np.zeros((N_CORES * NU_SHARD, N_CTRL), np.float64)
    Au[:N_EVAL] = _dense_basis(pu, p, N_CTRL)
    Av = _dense_basis(pv, p, N_CTRL)

    # host stage 1 (0.3% of the FLOPs): T[j, d, u] = sum_i P[i,j,d] Au[u,i]
    T = (cp.astype(np.float64).transpose(1, 2, 0).reshape(3 * N_CTRL, N_CTRL)
         @ Au.T).reshape(N_CTRL, 3, N_CORES * NU_SHARD)

    avt_hi, avt_lo = _split_bf16(Av.T.astype(np.float32))
    avt_hi = _dup_halves(avt_hi)
    avt_lo = _dup_halves(avt_lo)
    avt = {
        "avt_hi_a": np.ascontiguousarray(avt_hi[:, :V_SPLIT]),
        "avt_lo_a": np.ascontiguousarray(avt_lo[:, :V_SPLIT]),
        "avt_hi_b": np.ascontiguousarray(avt_hi[:, V_SPLIT:]),
        "avt_lo_b": np.ascontiguousarray(avt_lo[:, V_SPLIT:]),
    }

    nc = _build_device()
    in_maps = []
    for c in range(N_CORES):
        tt = T[:, :, c * NU_SHARD : (c + 1) * NU_SHARD].reshape(N_CTRL, -1)
        in_maps.append({"tt_p": _stack_hilo(tt.astype(np.float32)), **avt})

    res = run_bass_kernel_spmd(
        nc,
        in_maps,
        core_ids=list(range(N_CORES)),
        trace=_CACHE.get("trace", False),
        **_CACHE.get("run_kwargs", {}),
    )
    _CACHE["last_result"] = res
    full = np.concatenate([r["out"] for r in res.results], axis=0)[:N_EVAL]
    # d-plane rows -> [Nu, Nv, 3]
    return np.ascontiguousarray(
        full.reshape(N_EVAL, 3, N_EVAL).transpose(0, 2, 1)
    )



# revision 4
# speedup vs baseline: 1.6317x; 1.6317x over previous
"""B-spline (clamped) surface evaluation on 8 Trainium2 cores.

Math: out[u, v, :] = sum_{a,b} Bu[u,a] * Bv[v,b] * P[su[u]-p+a, sv[v]-p+b, :]

Host precomputes the tiny Cox-de-Boor basis, scatters it into dense matrices
Au [Nu, 64], Av [Nv, 64], and folds the small control-point contraction
T[u, j, d] = sum_i Au[u, i] P[i, j, d] (25M MACs, fp64 on host). The device
then does the dominant contraction (768M MACs):

  S[u, v, d] = sum_j T[u, j, d] * Av[v, j]       (TensorEngine matmuls)

The rel-err gate is 2e-2, so everything device-side runs in plain bf16
(~1e-3 total): no hi/lo split, and the output is written to HBM as bf16
(24 MB total instead of 48 MB) with the host casting back to fp32.

The K=64 contraction is zero-padded to K=128 (lhsT rows 64-127 = 0) so the
matmuls use the standard full-array config; matmul cycles scale with the
streamed column count, not K, so the padding is free.

Each (u-tile, d) group runs LDWEIGHTS once then streams 4 x N<=512 matmuls
into two 2-bank PSUM tiles; DVE and ACT alternate evacuating them with the
fp32->bf16 cast fused into wide [128, ~1000] copies (fp32 PSUM reads run at
1 elem/cycle/lane, so wide copies amortize the ~120-170 cycle fixed cost).
The group's [128, 2001] bf16 output region (512 KB) is flushed to HBM as
soon as its two copies land, round-robined over 4 SWDGE queues, so the
output DMA (the ~8.4 us/core roofline at 358 GB/s) overlaps compute instead
of draining after it.

Sharding: data-parallel over u. Each core computes a [251, 2001, 3] slab,
padded to 2x128 u-rows on device; the host drops the padding and
interleaves d.
"""

import numpy as np

N_CTRL = 64
N_EVAL = 2001
N_CORES = 8
NU_SHARD = 251   # ceil(2001 / 8); 8 * 251 = 2008 (last 7 rows are zero padding)
NU_PAD = 256     # per-core u padded to 2 full 128-wide PE column tiles
V_TILE = 512
V_HALF = 1024    # avt input splits here so the first matmuls start early

_CACHE = {}


def _clamped_knots(p, n_ctrl, dtype=np.float64):
    n_internal = n_ctrl - p - 1
    internal = np.linspace(0.0, 1.0, n_internal + 2, dtype=dtype)[1:-1]
    return np.concatenate(
        [np.zeros(p + 1, dtype), internal, np.ones(p + 1, dtype)]
    )


def _dense_basis(params, p, n_ctrl):
    """Dense basis matrix A [len(params), n_ctrl], float64, with
    A[k, span-p+a] = B[k, a] (Cox-de-Boor, NURBS book A2.2)."""
    knots = _clamped_knots(p, n_ctrl)
    u = np.asarray(params, np.float64)
    spans = np.clip(np.searchsorted(knots, u, side="right") - 1, p, n_ctrl - 1)
    Ns = [np.ones_like(u)]
    left = {}
    right = {}
    for j in range(1, p + 1):
        left[j] = u - knots[spans + 1 - j]
        right[j] = knots[spans + j] - u
        saved = np.zeros_like(u)
        new = []
        for r in range(j):
            temp = Ns[r] / (right[r + 1] + left[j - r])
            new.append(saved + right[r + 1] * temp)
            saved = left[j - r] * temp
        new.append(saved)
        Ns = new
    B = np.stack(Ns, axis=-1)  # [N, p+1]
    A = np.zeros((len(u), n_ctrl), np.float64)
    rows = np.arange(len(u))[:, None]
    cols = spans[:, None] - p + np.arange(p + 1)[None, :]
    A[rows, cols] = B
    return A


def _pad_k128(a_bf16):
    """[64, N] bf16 -> [128, N] with zeros in partitions 64-127."""
    return np.ascontiguousarray(
        np.concatenate([a_bf16, np.zeros_like(a_bf16)], axis=0)
    )


def _build_device():
    if "nc" in _CACHE:
        return _CACHE["nc"]

    import concourse.mybir as mybir
    import concourse.tile as tile
    from concourse import bacc

    f32 = mybir.dt.float32
    bf16 = mybir.dt.bfloat16
    nc = bacc.Bacc(
        "TRN2", target_bir_lowering=False, debug=False, num_devices=N_CORES,
        num_swdge_queues=4,
    )
    # ttz: [T; 0], cols d*NU_PAD + u.  avt_{a,b}: [Av.T half; 0]
    ins = {
        "ttz": nc.dram_tensor(
            "ttz", [128, 3 * NU_PAD], bf16, kind="ExternalInput"
        ).ap(),
        "avt_a": nc.dram_tensor(
            "avt_a", [128, V_HALF], bf16, kind="ExternalInput"
        ).ap(),
        "avt_b": nc.dram_tensor(
            "avt_b", [128, V_HALF], bf16, kind="ExternalInput"
        ).ap(),
    }
    # out col = g*6003 + d*2001 + v for u-tile g in {0, 1}
    out_h = nc.dram_tensor(
        "out", [128, 2 * 3 * N_EVAL], bf16, kind="ExternalOutput"
    ).ap()

    # (v0, width, avt half, col offset in half)
    VT = [
        (0, V_TILE, "avt_a", 0),
        (V_TILE, V_TILE, "avt_a", V_TILE),
        (V_HALF, V_TILE, "avt_b", 0),
        (V_HALF + V_TILE, N_EVAL - V_HALF - V_TILE, "avt_b", V_TILE),
    ]

    with tile.TileContext(nc) as tc:
        with (
            tc.tile_pool(name="consts", bufs=1) as consts,
            tc.tile_pool(name="ps", bufs=4, space="PSUM") as psp,
            tc.tile_pool(name="obuf", bufs=1) as obuf,
        ):
            sb = {}
            for name, cols in (("ttz", 3 * NU_PAD), ("avt_a", V_HALF),
                               ("avt_b", V_HALF)):
                sb[name] = consts.tile([128, cols], bf16, tag=name, name=name)
            for qi, name in ((0, "ttz"), (1, "avt_a"), (2, "avt_b")):
                d = nc.gpsimd.dma_start(out=sb[name], in_=ins[name])
                d.ins.queue = f"qPoolDynamic{qi or ''}"

            ob = {
                g: obuf.tile([128, 3 * N_EVAL], bf16, tag=f"ob{g}",
                             name=f"ob{g}")
                for g in range(2)
            }

            prev_mm = None
            n_out = 0
            gi = 0
            for d in range(3):
                for g in range(2):
                    csl = slice(d * NU_PAD + g * 128,
                                d * NU_PAD + g * 128 + 128)
                    w = sb["ttz"][:, csl]
                    ldw = nc.tensor.ldweights(w)
                    if prev_mm is not None:
                        tile.add_dep_helper(
                            ldw.ins, prev_mm.ins, sync=False,
                            reason="weight group order",
                        )
                    # two 2-bank psum tiles per group: v<1024 and v>=1024
                    for hi, (h0, hw) in enumerate(((0, V_HALF),
                                                   (V_HALF, N_EVAL - V_HALF))):
                        ps = psp.tile([128, V_HALF], f32, tag="ps")
                        for v0, vw, av, c0 in VT[2 * hi:2 * hi + 2]:
                            mm = nc.tensor.matmul(
                                ps[:, v0 - h0:v0 - h0 + vw], w,
                                sb[av][:, c0:c0 + vw],
                                start=True, stop=True,
                            )
                            mm.ins.ldweights = False
                            tile.add_dep_helper(
                                mm.ins, ldw.ins, sync=False,
                                reason="matmul after its ldweights",
                            )
                            prev_mm = mm
                        osl = slice(d * N_EVAL + h0, d * N_EVAL + h0 + hw)
                        # alternate engines; disjoint ranges run concurrently
                        if (hi == 0) == (gi % 2 == 0):
                            nc.vector.tensor_copy(ob[g][:, osl], ps[:, :hw])
                        else:
                            nc.scalar.copy(ob[g][:, osl], ps[:, :hw])
                    # flush this (u-tile, d) region once its copies land
                    osl = slice(d * N_EVAL, (d + 1) * N_EVAL)
                    dma = nc.gpsimd.dma_start(
                        out=out_h[:, g * 3 * N_EVAL + d * N_EVAL:
                                  g * 3 * N_EVAL + (d + 1) * N_EVAL],
                        in_=ob[g][:, osl],
                    )
                    qi = n_out % 4
                    dma.ins.queue = f"qPoolDynamic{qi or ''}"
                    n_out += 1
                    gi += 1
    nc.compile()
    _CACHE["nc"] = nc
    return nc


def kernel(control_points, params_u, params_v, degree):
    import ml_dtypes
    from concourse.bass_utils import run_bass_kernel_spmd

    p = int(np.asarray(degree))
    cp = np.asarray(control_points, np.float32)
    pu = np.asarray(params_u, np.float32)
    pv = np.asarray(params_v, np.float32)
    assert cp.shape == (N_CTRL, N_CTRL, 3), cp.shape
    assert pu.shape == (N_EVAL,) and pv.shape == (N_EVAL,), (pu.shape, pv.shape)

    Au = np.zeros((N_CORES * NU_SHARD, N_CTRL), np.float64)
    Au[:N_EVAL] = _dense_basis(pu, p, N_CTRL)
    Av = _dense_basis(pv, p, N_CTRL)

    # host stage 1 (0.3% of the FLOPs): T[j, d, u] = sum_i P[i,j,d] Au[u,i]
    T = (cp.astype(np.float64).transpose(1, 2, 0).reshape(3 * N_CTRL, N_CTRL)
         @ Au.T).reshape(N_CTRL, 3, N_CORES * NU_SHARD)

    avt = np.zeros((N_CTRL, 2 * V_HALF), np.float32)
    avt[:, :N_EVAL] = Av.T
    avt = avt.astype(ml_dtypes.bfloat16)
    avt_a = _pad_k128(avt[:, :V_HALF])
    avt_b = _pad_k128(avt[:, V_HALF:])

    nc = _build_device()
    in_maps = []
    for c in range(N_CORES):
        ttc = np.zeros((N_CTRL, 3, NU_PAD), np.float32)
        ttc[:, :, :NU_SHARD] = T[:, :, c * NU_SHARD:(c + 1) * NU_SHARD]
        ttz = _pad_k128(
            ttc.reshape(N_CTRL, 3 * NU_PAD).astype(ml_dtypes.bfloat16)
        )
        in_maps.append({"ttz": ttz, "avt_a": avt_a, "avt_b": avt_b})

    res = run_bass_kernel_spmd(
        nc,
        in_maps,
        core_ids=list(range(N_CORES)),
        trace=_CACHE.get("trace", False),
        **_CACHE.get("run_kwargs", {}),
    )
    _CACHE["last_result"] = res
    # out col = g*6003 + d*2001 + v; u-tile g=1 holds rows 128..250
    full = np.empty((N_CORES * NU_SHARD, 3, N_EVAL), np.float32)
    for c, r in enumerate(res.results):
        o = np.asarray(r["out"]).astype(np.float32)
        o = o.reshape(128, 2, 3, N_EVAL)  # cols are [g][d][v] row-major
        full[c * NU_SHARD:c * NU_SHARD + 128] = o[:, 0]
        full[c * NU_SHARD + 128:(c + 1) * NU_SHARD] = o[:NU_SHARD - 128, 1]
    return np.ascontiguousarray(full[:N_EVAL].transpose(0, 2, 1))
